# revision 1
# baseline (speedup 1.0000x reference)
"""Trainium2 Bass kernel for L0-regularized linear forward (hard-concrete gate).

Computes out[b,o] = sum_i x[b,i] * W[o,i] * z[b,o,i] + bias[o]
  where s = sigmoid((log(u) - log1p(-u) + log_alpha) / (2/3))
        z = clip(s * 1.2 - 0.1, 0, 1)

Shapes: x[32,2048] u[32,2048,2048] W[2048,2048] la[2048,2048] bias[2048]
Sharding: output-dim sharded, 2048/8 = 256 rows per core (each core reads
its slice of u/W/la/bias + full x; no collectives; concat outputs on host).

Per-core pipeline (o-tile layout [128 part, 2 halves, 2048 free]):
  ACT:  L1 = ln(u); L1 -= ln(1-u) via DVE; t = L1 + la (POOL); s = sigmoid(1.5 t)
  DVE:  z = clamp(1.2 s - 0.1, 0, 1); p = z * x_bcast;
        tensor_tensor_reduce: acc[o] = bias[o] + sum_i p * W   -> DMA to HBM
ACT table sets (ln vs sigmoid) are batched in groups of B_GROUP batches to
amortize the ~2.7us table switch.
"""

import sys
from contextlib import ExitStack

import numpy as np

if "/opt/trn_rl_repo" not in sys.path:
    sys.path.insert(0, "/opt/trn_rl_repo")

import concourse.bass as bass
import concourse.tile as tile
from concourse import bacc, mybir
from concourse.bass_utils import run_bass_kernel_spmd

F32 = mybir.dt.float32
F16 = mybir.dt.float16

B, OUT, IN = 32, 2048, 2048
N_CORES = 8
O_SH = OUT // N_CORES          # 256 output rows per core
H = O_SH // 128                # 2 partition-halves per core
B_GROUP = 8                    # batches per ACT-table-set phase

_CACHE = {}


def _build_nc(trace=False):
    key = ("nc", trace)
    if key in _CACHE:
        return _CACHE[key]

    nc = bacc.Bacc(
        "TRN2",
        target_bir_lowering=False,
        debug=False,
        num_devices=N_CORES,
    )
    x_d = nc.dram_tensor("x", [B, IN], F32, kind="ExternalInput").ap()
    u_d = nc.dram_tensor("u", [B, O_SH, IN], F32, kind="ExternalInput").ap()
    w_d = nc.dram_tensor("w", [O_SH, IN], F32, kind="ExternalInput").ap()
    la_d = nc.dram_tensor("la", [O_SH, IN], F32, kind="ExternalInput").ap()
    bias_d = nc.dram_tensor("bias", [O_SH], F32, kind="ExternalInput").ap()
    out_d = nc.dram_tensor("out", [B, O_SH], F32, kind="ExternalOutput").ap()

    with TileCtx(nc) as tc, ExitStack() as ctx:
        _kernel_body(ctx, tc, x_d, u_d, w_d, la_d, bias_d, out_d)

    nc.compile()
    _CACHE[key] = nc
    return nc


def TileCtx(nc):
    return tile.TileContext(nc)


def _bcast_row(ap_row):
    """[1, n] AP -> [128, n] AP with 0 partition stride."""
    return bass.AP(
        tensor=ap_row.tensor,
        offset=ap_row.offset,
        ap=[[0, 128], list(ap_row.ap[-1])],
    )


def _kernel_body(ctx, tc, x_d, u_d, w_d, la_d, bias_d, out_d):
    nc = tc.nc
    Ln = mybir.ActivationFunctionType.Ln
    Sig = mybir.ActivationFunctionType.Sigmoid
    op = mybir.AluOpType

    singles = ctx.enter_context(tc.tile_pool(name="singles", bufs=1))

    # --- constants: W, la as f16 [128, H, IN]; bias cols; x16 rows ---
    w16 = singles.tile([128, H, IN], F16)
    la16 = singles.tile([128, H, IN], F16)
    with tc.tile_pool(name="setup", bufs=1) as setup:
        w32 = setup.tile([128, H, IN], F32)
        nc.sync.dma_start(out=w32, in_=w_d.rearrange("(h p) i -> p h i", p=128))
        nc.vector.tensor_copy(w16, w32)
        la32 = setup.tile([128, H, IN], F32)
        nc.sync.dma_start(out=la32, in_=la_d.rearrange("(h p) i -> p h i", p=128))
        nc.vector.tensor_copy(la16, la32)

    x16_hbm = nc.dram_tensor("x16tmp", [B, IN], F16, kind="Internal").ap()
    with tc.tile_pool(name="setup2", bufs=1) as setup:
        x32 = setup.tile([B, IN], F32)
        nc.sync.dma_start(out=x32, in_=x_d)
        x16 = setup.tile([B, IN], F16)
        nc.vector.tensor_copy(x16, x32)
        nc.sync.dma_start(out=x16_hbm, in_=x16)

    bias_col = singles.tile([128, H], F32)
    nc.sync.dma_start(out=bias_col, in_=bias_d.rearrange("(h p) -> p h", p=128))

    # --- pools for the main loop ---
    upool = ctx.enter_context(tc.tile_pool(name="u", bufs=2))
    l1pool = ctx.enter_context(tc.tile_pool(name="l1", bufs=2))
    l2pool = ctx.enter_context(tc.tile_pool(name="l2", bufs=2))
    tpool = ctx.enter_context(tc.tile_pool(name="t", bufs=B_GROUP + 1))
    zpool = ctx.enter_context(tc.tile_pool(name="z", bufs=3))
    xbpool = ctx.enter_context(tc.tile_pool(name="xb", bufs=3))
    ppool = ctx.enter_context(tc.tile_pool(name="p", bufs=4))
    apool = ctx.enter_context(tc.tile_pool(name="acc", bufs=8))

    out_v = out_d.rearrange("b (h p) -> b p h", p=128)

    for g0 in range(0, B, B_GROUP):
        grp = range(g0, min(g0 + B_GROUP, B))
        t_tiles = {}
        # ---- phase 1: natural_log table set ----
        for b in grp:
            ut = upool.tile([128, H, IN], F32)
            nc.sync.dma_start(
                out=ut, in_=u_d[b].rearrange("(h p) i -> p h i", p=128)
            )
            l1 = l1pool.tile([128, H, IN], F16)
            nc.scalar.activation(l1, ut, Ln)                      # ln(u)
            l2 = l2pool.tile([128, H, IN], F16)
            nc.scalar.activation(l2, ut, Ln, bias=1.0, scale=-1.0)  # ln(1-u)
            nc.vector.tensor_sub(l1, l1, l2)                      # logit(u), in place
            t16 = tpool.tile([128, H, IN], F16)
            nc.gpsimd.tensor_add(t16, l1, la16)                   # + log_alpha
            t_tiles[b] = t16
        # ---- phase 2: sigmoid table set ----
        for b in grp:
            t16 = t_tiles[b]
            nc.scalar.activation(t16, t16, Sig, scale=1.5)        # s, in place
            z16 = zpool.tile([128, H, IN], F16)
            nc.vector.tensor_scalar(z16, t16, 1.2, -0.1, op.mult, op.add)
            nc.vector.tensor_scalar(z16, z16, 0.0, 1.0, op.max, op.min)
            xb = xbpool.tile([128, IN], F16)
            nc.sync.dma_start(out=xb, in_=_bcast_row(x16_hbm[b : b + 1, :]))
            for h in range(H):
                p16 = ppool.tile([128, IN], F16)
                nc.vector.tensor_mul(p16, z16[:, h, :], xb)
                acc = apool.tile([128, 1], F32)
                nc.vector.scalar_tensor_tensor(
                    out=p16,
                    in0=p16,
                    scalar=1.0,
                    in1=w16[:, h, :],
                    op0=op.bypass,
                    op1=op.mult,
                    accum_out=acc,
                )
                nc.vector.tensor_add(acc, acc, bias_col[:, h : h + 1])
                nc.sync.dma_start(out=out_v[b, :, h : h + 1], in_=acc)


def kernel(x, u, weight, log_alpha, bias):
    x = np.ascontiguousarray(x, dtype=np.float32)
    u = np.ascontiguousarray(u, dtype=np.float32)
    weight = np.ascontiguousarray(weight, dtype=np.float32)
    log_alpha = np.ascontiguousarray(log_alpha, dtype=np.float32)
    bias = np.ascontiguousarray(bias, dtype=np.float32)

    nc = _build_nc()

    in_maps = []
    for c in range(N_CORES):
        sl = slice(c * O_SH, (c + 1) * O_SH)
        in_maps.append(
            {
                "x": x,
                "u": np.ascontiguousarray(u[:, sl, :]),
                "w": np.ascontiguousarray(weight[sl]),
                "la": np.ascontiguousarray(log_alpha[sl]),
                "bias": np.ascontiguousarray(bias[sl]),
            }
        )

    import os

    trace = bool(int(os.environ.get("KERNEL_TRACE", "0")))
    res = run_bass_kernel_spmd(
        nc, in_maps, core_ids=list(range(N_CORES)), trace=trace
    )
    kernel._last = res

    out = np.empty((B, OUT), dtype=np.float32)
    for c in range(N_CORES):
        out[:, c * O_SH : (c + 1) * O_SH] = res.results[c]["out"]
    return out



# revision 11
# speedup vs baseline: 1.7069x; 1.7069x over previous
"""Trainium2 Bass kernel for L0-regularized linear forward (hard-concrete gate).

Computes out[b,o] = sum_i x[b,i] * W[o,i] * z[b,o,i] + bias[o]
  where s = sigmoid((log(u) - log1p(-u) + log_alpha) / (2/3))
        z = clip(s * 1.2 - 0.1, 0, 1)

Shapes: x[32,2048] u[32,2048,2048] W[2048,2048] la[2048,2048] bias[2048]
Sharding: output-dim sharded, 2048/8 = 256 rows per core; no collectives.

Math used on device:
  y  = ln(u) - ln(1-u) + la;  yc = clamp(y, +-c), c = ln(11)/1.5
  z  = 1.2*sigmoid(1.5*yc) - 0.1              (exact: clip folds into clamp)
  sigmoid(1.5*y)-0.5 ~= p(y) = (c3*y^2 + c1)*y (deg-3, coefficients optimized
  for end-to-end variance against u~U(0,1); rel out err ~6e-3 vs 2e-2 gate)
  out = sum_i p(yc)*cb + 0.5*S0 + bias   with cb = 1.2*w*x, S0 = sum_i w*x

Engine schedule per (batch, half-of-256-rows) unit, all f16 after ACT:
  DMA : u half-tile (f32), broadcast row of x (f16)
  ACT : l2 = ln(1-u), l1 = ln(u)       (only the natural_log table -> 1 load)
  Pool: y1 = la - l2                   (scalar_tensor_tensor, eff 0.6)
  DVE : y = l1 + y1; custom op HC_GATE3_RED = clamp+poly3+mul-cb+accumulate
        in ONE instruction; cb = w12*xb (some offloaded to Pool)
  PE  : S0 row-sums of w*x (tiny f16 matmuls into PSUM)
"""

import sys
from contextlib import ExitStack

import numpy as np

if "/opt/trn_rl_repo" not in sys.path:
    sys.path.insert(0, "/opt/trn_rl_repo")

import concourse.bass as bass
import concourse.tile as tile
from concourse import bacc, mybir
from concourse.bass_utils import run_bass_kernel_spmd

F32 = mybir.dt.float32
F16 = mybir.dt.float16

B, OUT, IN = 32, 2048, 2048
N_CORES = 8
O_SH = OUT // N_CORES          # 256 output rows per core
H = O_SH // 128                # 2 partition-halves per core

C_CLAMP = float(np.log(11.0) / 1.5)
# variance-optimal deg-3: sigmoid(1.5 y) - 0.5 ~ (C3 y^2 + C1) y on [-c, c]
C1, C3 = 0.358500, -0.038292

N_CB_POOL = 3                  # of every 8 cb passes, this many go to Pool

_CACHE = {}


def _register_custom_op():
    """One fused DVE instruction:
        yc  = clamp(in0, -s0, s0)
        out = ((yc^2 * s1 + imm2) * yc) * in1
        accum_out = sum(out, free axis)
    8 ALU stages exactly (Zero-C0 is hoisted as stream-invariant)."""
    from concourse.dve_ops import (
        CUSTOM_DVE_SPECS,
        OPS,
        _CUSTOM_DVE_ROW_BASE,
        _SUB_OPCODE_FOR_NAME,
        DveOp,
    )
    from concourse.dve_spec import (
        AluOp, C0, Spec, Src0, Src1, Zero, lower, maxx, minn, sq,
    )
    from concourse.dve_spec import C1 as C1L, C2 as C2L
    from concourse.dve_table_gen import DveOpSpec

    name = "HC_GATE3_RED"
    if name in CUSTOM_DVE_SPECS:
        return next(o for o in OPS if o.name == name)

    yc = maxx(minn(Src0, C0), Zero - C0)
    body = ((sq(yc) * C1L + C2L) * yc) * Src1

    def _ref(in0, in1, s0, s1, imm2):
        x = np.clip(np.asarray(in0, np.float32), -s0, s0)
        w = np.asarray(in1, np.float32)
        out = ((x * x * s1 + imm2) * x) * w
        return out, out.sum(axis=-1, keepdims=True)

    spec = Spec(body=body, accum=AluOp.ADD, reference=_ref)
    row = _CUSTOM_DVE_ROW_BASE + len(OPS)
    _SUB_OPCODE_FOR_NAME[name] = row
    shas = {}
    for ver in ("v3", "v4"):
        dspec = DveOpSpec(name=name, opcode=row, uops=lower(spec, ver=ver),
                          rd1_en=True)
        shas[ver] = dspec.sha(ver)
    dve_op = DveOp(name, spec, subdim=False, uops_sha=shas)
    OPS.append(dve_op)
    CUSTOM_DVE_SPECS[name] = spec
    return dve_op


def _build_nc():
    if "nc" in _CACHE:
        return _CACHE["nc"]

    dve_op = _register_custom_op()

    nc = bacc.Bacc(
        "TRN2",
        target_bir_lowering=False,
        debug=False,
        num_devices=N_CORES,
    )
    u_d = nc.dram_tensor("u", [B, O_SH, IN], F32, kind="ExternalInput").ap()
    x16_d = nc.dram_tensor("x16", [B, IN], F16, kind="ExternalInput").ap()
    w12_d = nc.dram_tensor("w12", [O_SH, IN], F16, kind="ExternalInput").ap()
    la16_d = nc.dram_tensor("la16", [O_SH, IN], F16, kind="ExternalInput").ap()
    wt16_d = nc.dram_tensor("wt16", [IN, O_SH], F16, kind="ExternalInput").ap()
    xt16_d = nc.dram_tensor("xt16", [IN, B], F16, kind="ExternalInput").ap()
    bias_d = nc.dram_tensor("bias", [O_SH], F32, kind="ExternalInput").ap()
    out_d = nc.dram_tensor("out", [128, H, B], F32, kind="ExternalOutput").ap()

    with tile.TileContext(nc) as tc, ExitStack() as ctx:
        _kernel_body(ctx, tc, dve_op, u_d, x16_d, w12_d, la16_d, wt16_d,
                     xt16_d, bias_d, out_d)

    nc.compile()
    _CACHE["nc"] = nc
    return nc


def _bcast_row(ap_row):
    """[1, n] AP -> [128, n] AP with 0 partition stride."""
    return bass.AP(
        tensor=ap_row.tensor,
        offset=ap_row.offset,
        ap=[[0, 128], list(ap_row.ap[-1])],
    )


def _kernel_body(ctx, tc, dve_op, u_d, x16_d, w12_d, la16_d, wt16_d, xt16_d,
                 bias_d, out_d):
    nc = tc.nc
    Ln = mybir.ActivationFunctionType.Ln
    op = mybir.AluOpType

    singles = ctx.enter_context(tc.tile_pool(name="singles", bufs=1))

    # --- constants ---
    w12 = singles.tile([128, H, IN], F16)
    nc.sync.dma_start(out=w12, in_=w12_d.rearrange("(h p) i -> p h i", p=128))
    la16 = singles.tile([128, H, IN], F16)
    nc.sync.dma_start(out=la16, in_=la16_d.rearrange("(h p) i -> p h i", p=128))
    wt = singles.tile([128, IN // 128, O_SH], F16)
    nc.sync.dma_start(out=wt, in_=wt16_d.rearrange("(ki p) o -> p ki o", p=128))
    xt = singles.tile([128, IN // 128, B], F16)
    nc.sync.dma_start(out=xt, in_=xt16_d.rearrange("(ki p) b -> p ki b", p=128))
    bias_col = singles.tile([128, H], F32)
    nc.sync.dma_start(out=bias_col, in_=bias_d.rearrange("(h p) -> p h", p=128))

    # accumulator strip, column index = h*B + b
    accM = singles.tile([128, H * B], F32)
    s0 = singles.tile([128, H * B], F32)

    # --- S0 via PE: S0[o, b] = sum_i w[o,i] x[b,i] ---
    with tc.psum_pool(name="ps", bufs=2) as psp:
        for h in range(H):
            osl = slice(h * 128, (h + 1) * 128)
            pm = psp.tile([128, B], F32)
            for ki in range(IN // 128):
                nc.tensor.matmul(pm, wt[:, ki, osl], xt[:, ki, :],
                                 start=(ki == 0), stop=(ki == IN // 128 - 1))
            nc.vector.tensor_copy(s0[:, h * B:(h + 1) * B], pm)

    # --- main loop pools ---
    upool = ctx.enter_context(tc.tile_pool(name="u", bufs=4))
    l2pool = ctx.enter_context(tc.tile_pool(name="l2", bufs=4))
    ycpool = ctx.enter_context(tc.tile_pool(name="yc", bufs=6))
    xbpool = ctx.enter_context(tc.tile_pool(name="xb", bufs=3))
    cbpool = ctx.enter_context(tc.tile_pool(name="cb", bufs=6))
    junkpool = ctx.enter_context(tc.tile_pool(name="junk", bufs=2))

    idx = 0
    for b in range(B):
        xb = xbpool.tile([128, IN], F16)
        nc.sync.dma_start(out=xb, in_=_bcast_row(x16_d[b:b + 1, :]))
        for h in range(H):
            ut = upool.tile([128, IN], F32)
            nc.sync.dma_start(out=ut, in_=u_d[b, h * 128:(h + 1) * 128, :])
            l2 = l2pool.tile([128, IN], F16)
            nc.scalar.activation(l2, ut, Ln, bias=1.0, scale=-1.0)   # ln(1-u)
            # DVE: y1 = la - l2 (in place, plain tensor_tensor runs at 2x)
            nc.vector.tensor_sub(l2, la16[:, h, :], l2)
            yc = ycpool.tile([128, IN], F16)
            nc.scalar.activation(yc, ut, Ln)                         # ln(u)
            nc.vector.tensor_add(yc, yc, l2)                         # y

            cb = cbpool.tile([128, IN], F16)
            nc.gpsimd.tensor_mul(cb, w12[:, h, :], xb)               # Pool
            idx += 1

            junk = junkpool.tile([128, IN], F16)
            col = h * B + b
            nc.vector._custom_dve(
                dve_op,
                out=junk,
                in0=yc,
                in1=cb,
                s0=C_CLAMP, s1=C3, imm2=C1,
                accum_out=accM[:, col:col + 1],
            )

    # --- final combine: out = accM + 0.5*S0 + bias ---
    comb = singles.tile([128, H * B], F32)
    nc.vector.scalar_tensor_tensor(out=comb, in0=s0, scalar=0.5,
                                   in1=accM, op0=op.mult, op1=op.add)
    for h in range(H):
        nc.vector.tensor_scalar(
            comb[:, h * B:(h + 1) * B], comb[:, h * B:(h + 1) * B],
            bias_col[:, h:h + 1], None, op.add,
        )
    out_v = out_d.rearrange("p h b -> p (h b)")
    nc.sync.dma_start(out=out_v, in_=comb)


def kernel(x, u, weight, log_alpha, bias):
    x = np.ascontiguousarray(x, dtype=np.float32)
    u = np.ascontiguousarray(u, dtype=np.float32)
    weight = np.ascontiguousarray(weight, dtype=np.float32)
    log_alpha = np.ascontiguousarray(log_alpha, dtype=np.float32)
    bias = np.ascontiguousarray(bias, dtype=np.float32)

    nc = _build_nc()

    x16 = x.astype(np.float16)
    in_maps = []
    for c in range(N_CORES):
        sl = slice(c * O_SH, (c + 1) * O_SH)
        wsl = weight[sl]
        in_maps.append(
            {
                "u": np.ascontiguousarray(u[:, sl, :]),
                "x16": x16,
                "w12": np.ascontiguousarray((1.2 * wsl).astype(np.float16)),
                "la16": np.ascontiguousarray(log_alpha[sl].astype(np.float16)),
                "wt16": np.ascontiguousarray(wsl.T.astype(np.float16)),
                "xt16": np.ascontiguousarray(x.T.astype(np.float16)),
                "bias": np.ascontiguousarray(bias[sl]),
            }
        )

    import os

    trace = bool(int(os.environ.get("KERNEL_TRACE", "0")))
    res = run_bass_kernel_spmd(
        nc, in_maps, core_ids=list(range(N_CORES)), trace=trace
    )
    kernel._last = res

    out = np.empty((B, OUT), dtype=np.float32)
    for c in range(N_CORES):
        oc = res.results[c]["out"]          # [128, H, B]
        out[:, c * O_SH:(c + 1) * O_SH] = oc.transpose(2, 1, 0).reshape(B, O_SH)
    return out


# revision 16
# speedup vs baseline: 1.7149x; 1.0047x over previous
"""Trainium2 Bass kernel for L0-regularized linear forward (hard-concrete gate).

Computes out[b,o] = sum_i x[b,i] * W[o,i] * z[b,o,i] + bias[o]
  where s = sigmoid((log(u) - log1p(-u) + log_alpha) / (2/3))
        z = clip(s * 1.2 - 0.1, 0, 1)

Shapes: x[32,2048] u[32,2048,2048] W[2048,2048] la[2048,2048] bias[2048]
Sharding: output-dim sharded, 2048/8 = 256 rows per core; no collectives.

Math used on device:
  y  = ln(u) - ln(1-u) + la;  yc = clamp(y, +-c), c = ln(11)/1.5
  z  = 1.2*sigmoid(1.5*yc) - 0.1              (exact: clip folds into clamp)
  sigmoid(1.5*y)-0.5 ~= p(y) = (c3*y^2 + c1)*y (deg-3, coefficients optimized
  for end-to-end variance against u~U(0,1); rel out err ~6e-3 vs 2e-2 gate)
  out = sum_i p(yc)*cb + 0.5*S0 + bias   with cb = 1.2*w*x, S0 = sum_i w*x

Engine schedule per (batch, half-of-256-rows) unit, all f16 after ACT:
  DMA : u half-tile (f32), broadcast row of x (f16)
  ACT : l2 = ln(1-u), l1 = ln(u)       (only the natural_log table -> 1 load)
  Pool: y1 = la - l2                   (scalar_tensor_tensor, eff 0.6)
  DVE : y = l1 + y1; custom op HC_GATE3_RED = clamp+poly3+mul-cb+accumulate
        in ONE instruction; cb = w12*xb (some offloaded to Pool)
  PE  : S0 row-sums of w*x (tiny f16 matmuls into PSUM)
"""

import sys
from contextlib import ExitStack

import numpy as np

if "/opt/trn_rl_repo" not in sys.path:
    sys.path.insert(0, "/opt/trn_rl_repo")

import concourse.bass as bass
import concourse.tile as tile
from concourse import bacc, mybir
from concourse.bass_utils import run_bass_kernel_spmd

F32 = mybir.dt.float32
F16 = mybir.dt.float16

B, OUT, IN = 32, 2048, 2048
N_CORES = 8
O_SH = OUT // N_CORES          # 256 output rows per core
H = O_SH // 128                # 2 partition-halves per core

C_CLAMP = float(np.log(11.0) / 1.5)
# variance-optimal deg-3: sigmoid(1.5 y) - 0.5 ~ (C3 y^2 + C1) y on [-c, c]
C1, C3 = 0.358500, -0.038292

N_CB_POOL = 3                  # of every 8 cb passes, this many go to Pool

_CACHE = {}


def _register_custom_op():
    """One fused DVE instruction:
        yc  = clamp(in0, -s0, s0)
        out = ((yc^2 * s1 + imm2) * yc) * in1
        accum_out = sum(out, free axis)
    8 ALU stages exactly (Zero-C0 is hoisted as stream-invariant)."""
    from concourse.dve_ops import (
        CUSTOM_DVE_SPECS,
        OPS,
        _CUSTOM_DVE_ROW_BASE,
        _SUB_OPCODE_FOR_NAME,
        DveOp,
    )
    from concourse.dve_spec import (
        AluOp, C0, Spec, Src0, Src1, Zero, lower, maxx, minn, sq,
    )
    from concourse.dve_spec import C1 as C1L, C2 as C2L
    from concourse.dve_table_gen import DveOpSpec

    name = "HC_GATE3_RED"
    if name in CUSTOM_DVE_SPECS:
        return next(o for o in OPS if o.name == name)

    yc = maxx(minn(Src0, C0), Zero - C0)
    body = ((sq(yc) * C1L + C2L) * yc) * Src1

    def _ref(in0, in1, s0, s1, imm2):
        x = np.clip(np.asarray(in0, np.float32), -s0, s0)
        w = np.asarray(in1, np.float32)
        out = ((x * x * s1 + imm2) * x) * w
        return out, out.sum(axis=-1, keepdims=True)

    spec = Spec(body=body, accum=AluOp.ADD, reference=_ref)
    row = _CUSTOM_DVE_ROW_BASE + len(OPS)
    _SUB_OPCODE_FOR_NAME[name] = row
    shas = {}
    for ver in ("v3", "v4"):
        dspec = DveOpSpec(name=name, opcode=row, uops=lower(spec, ver=ver),
                          rd1_en=True)
        shas[ver] = dspec.sha(ver)
    dve_op = DveOp(name, spec, subdim=False, uops_sha=shas)
    OPS.append(dve_op)
    CUSTOM_DVE_SPECS[name] = spec
    return dve_op


def _build_nc():
    if "nc" in _CACHE:
        return _CACHE["nc"]

    dve_op = _register_custom_op()

    nc = bacc.Bacc(
        "TRN2",
        target_bir_lowering=False,
        debug=False,
        num_devices=N_CORES,
    )
    u_d = nc.dram_tensor("u", [B, O_SH, IN], F32, kind="ExternalInput").ap()
    x16_d = nc.dram_tensor("x16", [B, IN], F16, kind="ExternalInput").ap()
    w12_d = nc.dram_tensor("w12", [O_SH, IN], F16, kind="ExternalInput").ap()
    la16_d = nc.dram_tensor("la16", [O_SH, IN], F16, kind="ExternalInput").ap()
    wt16_d = nc.dram_tensor("wt16", [IN, O_SH], F16, kind="ExternalInput").ap()
    xt16_d = nc.dram_tensor("xt16", [IN, B], F16, kind="ExternalInput").ap()
    bias_d = nc.dram_tensor("bias", [O_SH], F32, kind="ExternalInput").ap()
    out_d = nc.dram_tensor("out", [128, H, B], F32, kind="ExternalOutput").ap()

    with tile.TileContext(nc) as tc, ExitStack() as ctx:
        _kernel_body(ctx, tc, dve_op, u_d, x16_d, w12_d, la16_d, wt16_d,
                     xt16_d, bias_d, out_d)

    nc.compile()
    _CACHE["nc"] = nc
    return nc


def _bcast_row(ap_row):
    """[1, n] AP -> [128, n] AP with 0 partition stride."""
    return bass.AP(
        tensor=ap_row.tensor,
        offset=ap_row.offset,
        ap=[[0, 128], list(ap_row.ap[-1])],
    )


def _kernel_body(ctx, tc, dve_op, u_d, x16_d, w12_d, la16_d, wt16_d, xt16_d,
                 bias_d, out_d):
    nc = tc.nc
    Ln = mybir.ActivationFunctionType.Ln
    op = mybir.AluOpType

    singles = ctx.enter_context(tc.tile_pool(name="singles", bufs=1))

    # --- main loop pools (declared before constants so the first u/xb DMAs
    # are issued ahead of the setup DMAs on the in-order DMA queue) ---
    upool = ctx.enter_context(tc.tile_pool(name="u", bufs=2))
    l2pool = ctx.enter_context(tc.tile_pool(name="l2", bufs=2))
    ycpool = ctx.enter_context(tc.tile_pool(name="yc", bufs=3))
    xbpool = ctx.enter_context(tc.tile_pool(name="xb", bufs=3))
    cbpool = ctx.enter_context(tc.tile_pool(name="cb", bufs=6))
    junkpool = ctx.enter_context(tc.tile_pool(name="junk", bufs=2))

    # first working-set DMAs go out first; batch 0 is loaded per half so the
    # first ACT/DVE work starts after ~3us instead of ~6us of DMA
    ut0 = upool.tile([128, H, IN], F32)
    la16 = singles.tile([128, H, IN], F16)
    w12 = singles.tile([128, H, IN], F16)
    xb0 = xbpool.tile([128, IN], F16)
    la_v = la16_d.rearrange("(h p) i -> p h i", p=128)
    w12_v = w12_d.rearrange("(h p) i -> p h i", p=128)
    # dependency-ordered first transfers: everything the first (b=0, h=0)
    # unit needs, then the h=1 half, then the rest
    nc.sync.dma_start(out=ut0[:, 0, :], in_=u_d[0, 0:128, :])
    nc.sync.dma_start(out=la16[:, 0, :], in_=la_v[:, 0, :])
    nc.sync.dma_start(out=xb0, in_=_bcast_row(x16_d[0:1, :]))
    nc.sync.dma_start(out=w12[:, 0, :], in_=w12_v[:, 0, :])
    nc.sync.dma_start(out=ut0[:, 1, :], in_=u_d[0, 128:256, :])
    nc.sync.dma_start(out=la16[:, 1, :], in_=la_v[:, 1, :])
    nc.sync.dma_start(out=w12[:, 1, :], in_=w12_v[:, 1, :])
    bias_col = singles.tile([128, H], F32)
    nc.sync.dma_start(out=bias_col, in_=bias_d.rearrange("(h p) -> p h", p=128))

    # accumulator strip, column index = h*B + b
    accM = singles.tile([128, H * B], F32)
    s0 = singles.tile([128, H * B], F32)

    def unit(b, ut, xb):
        l2 = l2pool.tile([128, H, IN], F16)
        yc = ycpool.tile([128, H, IN], F16)
        if b == 0:
            # per-half for a shorter pipeline-fill ramp
            for h in range(H):
                nc.scalar.activation(l2[:, h, :], ut[:, h, :], Ln,
                                     bias=1.0, scale=-1.0)
                nc.vector.tensor_sub(l2[:, h, :], la16[:, h, :], l2[:, h, :])
                nc.scalar.activation(yc[:, h, :], ut[:, h, :], Ln)
                nc.vector.tensor_add(yc[:, h, :], yc[:, h, :], l2[:, h, :])
        else:
            nc.scalar.activation(l2, ut, Ln, bias=1.0, scale=-1.0)   # ln(1-u)
            # DVE: y1 = la - l2 (in place, plain tensor_tensor runs at 2x)
            nc.vector.tensor_sub(l2, la16, l2)
            nc.scalar.activation(yc, ut, Ln)                         # ln(u)
            nc.vector.tensor_add(yc, yc, l2)                         # y
        for h in range(H):
            cb = cbpool.tile([128, IN], F16)
            nc.gpsimd.tensor_mul(cb, w12[:, h, :], xb)           # Pool
            junk = junkpool.tile([128, IN], F16)
            col = h * B + b
            nc.vector._custom_dve(
                dve_op,
                out=junk,
                in0=yc[:, h, :],
                in1=cb,
                s0=C_CLAMP, s1=C3, imm2=C1,
                accum_out=accM[:, col:col + 1],
            )

    for b in range(B):
        ut = ut0 if b == 0 else upool.tile([128, H, IN], F32)
        if b > 0:
            nc.sync.dma_start(out=ut, in_=u_d[b].rearrange("(h p) i -> p h i", p=128))
        xb = xb0 if b == 0 else xbpool.tile([128, IN], F16)
        if b > 0:
            nc.sync.dma_start(out=xb, in_=_bcast_row(x16_d[b:b + 1, :]))
        unit(b, ut, xb)

    # --- S0 via PE: S0[o, b] = sum_i w[o,i] x[b,i] (feeds only the combine,
    # so it is emitted last and fills engine idle time near the tail) ---
    wt = singles.tile([128, IN // 128, O_SH], F16)
    nc.sync.dma_start(out=wt, in_=wt16_d.rearrange("(ki p) o -> p ki o", p=128))
    xt = singles.tile([128, IN // 128, B], F16)
    nc.sync.dma_start(out=xt, in_=xt16_d.rearrange("(ki p) b -> p ki b", p=128))
    with tc.psum_pool(name="ps", bufs=2) as psp:
        for h in range(H):
            osl = slice(h * 128, (h + 1) * 128)
            pm = psp.tile([128, B], F32)
            for ki in range(IN // 128):
                nc.tensor.matmul(pm, wt[:, ki, osl], xt[:, ki, :],
                                 start=(ki == 0), stop=(ki == IN // 128 - 1))
            nc.vector.tensor_copy(s0[:, h * B:(h + 1) * B], pm)

    # --- final combine: out = accM + 0.5*S0 + bias ---
    comb = singles.tile([128, H * B], F32)
    nc.vector.scalar_tensor_tensor(out=comb, in0=s0, scalar=0.5,
                                   in1=accM, op0=op.mult, op1=op.add)
    for h in range(H):
        nc.vector.tensor_scalar(
            comb[:, h * B:(h + 1) * B], comb[:, h * B:(h + 1) * B],
            bias_col[:, h:h + 1], None, op.add,
        )
    out_v = out_d.rearrange("p h b -> p (h b)")
    nc.sync.dma_start(out=out_v, in_=comb)


def kernel(x, u, weight, log_alpha, bias):
    x = np.ascontiguousarray(x, dtype=np.float32)
    u = np.ascontiguousarray(u, dtype=np.float32)
    weight = np.ascontiguousarray(weight, dtype=np.float32)
    log_alpha = np.ascontiguousarray(log_alpha, dtype=np.float32)
    bias = np.ascontiguousarray(bias, dtype=np.float32)

    nc = _build_nc()

    x16 = x.astype(np.float16)
    in_maps = []
    for c in range(N_CORES):
        sl = slice(c * O_SH, (c + 1) * O_SH)
        wsl = weight[sl]
        in_maps.append(
            {
                "u": np.ascontiguousarray(u[:, sl, :]),
                "x16": x16,
                "w12": np.ascontiguousarray((1.2 * wsl).astype(np.float16)),
                "la16": np.ascontiguousarray(log_alpha[sl].astype(np.float16)),
                "wt16": np.ascontiguousarray(wsl.T.astype(np.float16)),
                "xt16": np.ascontiguousarray(x.T.astype(np.float16)),
                "bias": np.ascontiguousarray(bias[sl]),
            }
        )

    import os

    trace = bool(int(os.environ.get("KERNEL_TRACE", "0")))
    res = run_bass_kernel_spmd(
        nc, in_maps, core_ids=list(range(N_CORES)), trace=trace
    )
    kernel._last = res

    out = np.empty((B, OUT), dtype=np.float32)
    for c in range(N_CORES):
        oc = res.results[c]["out"]          # [128, H, B]
        out[:, c * O_SH:(c + 1) * O_SH] = oc.transpose(2, 1, 0).reshape(B, O_SH)
    return out


# revision 17
# speedup vs baseline: 1.7230x; 1.0047x over previous
"""Trainium2 Bass kernel for L0-regularized linear forward (hard-concrete gate).

Computes out[b,o] = sum_i x[b,i] * W[o,i] * z[b,o,i] + bias[o]
  where s = sigmoid((log(u) - log1p(-u) + log_alpha) / (2/3))
        z = clip(s * 1.2 - 0.1, 0, 1)

Shapes: x[32,2048] u[32,2048,2048] W[2048,2048] la[2048,2048] bias[2048]
Sharding: output-dim sharded, 2048/8 = 256 rows per core; no collectives.

Math used on device:
  y  = ln(u) - ln(1-u) + la;  yc = clamp(y, +-c), c = ln(11)/1.5
  z  = 1.2*sigmoid(1.5*yc) - 0.1              (exact: clip folds into clamp)
  sigmoid(1.5*y)-0.5 ~= p(y) = (c3*y^2 + c1)*y (deg-3, coefficients optimized
  for end-to-end variance against u~U(0,1); rel out err ~6e-3 vs 2e-2 gate)
  out = sum_i p(yc)*cb + 0.5*S0 + bias   with cb = 1.2*w*x, S0 = sum_i w*x

Engine schedule per (batch, half-of-256-rows) unit, all f16 after ACT:
  DMA : u half-tile (f32), broadcast row of x (f16)
  ACT : l2 = ln(1-u), l1 = ln(u)       (only the natural_log table -> 1 load)
  Pool: y1 = la - l2                   (scalar_tensor_tensor, eff 0.6)
  DVE : y = l1 + y1; custom op HC_GATE3_RED = clamp+poly3+mul-cb+accumulate
        in ONE instruction; cb = w12*xb (some offloaded to Pool)
  PE  : S0 row-sums of w*x (tiny f16 matmuls into PSUM)
"""

import sys
from contextlib import ExitStack

import numpy as np

if "/opt/trn_rl_repo" not in sys.path:
    sys.path.insert(0, "/opt/trn_rl_repo")

import concourse.bass as bass
import concourse.tile as tile
from concourse import bacc, mybir
from concourse.bass_utils import run_bass_kernel_spmd

F32 = mybir.dt.float32
F16 = mybir.dt.float16

B, OUT, IN = 32, 2048, 2048
N_CORES = 8
O_SH = OUT // N_CORES          # 256 output rows per core
H = O_SH // 128                # 2 partition-halves per core

C_CLAMP = float(np.log(11.0) / 1.5)
# variance-optimal deg-3: sigmoid(1.5 y) - 0.5 ~ (C3 y^2 + C1) y on [-c, c]
C1, C3 = 0.358500, -0.038292

N_CB_POOL = 3                  # of every 8 cb passes, this many go to Pool

_CACHE = {}


def _register_custom_op():
    """One fused DVE instruction:
        yc  = clamp(in0, -s0, s0)
        out = ((yc^2 * s1 + imm2) * yc) * in1
        accum_out = sum(out, free axis)
    8 ALU stages exactly (Zero-C0 is hoisted as stream-invariant)."""
    from concourse.dve_ops import (
        CUSTOM_DVE_SPECS,
        OPS,
        _CUSTOM_DVE_ROW_BASE,
        _SUB_OPCODE_FOR_NAME,
        DveOp,
    )
    from concourse.dve_spec import (
        AluOp, C0, Spec, Src0, Src1, Zero, lower, maxx, minn, sq,
    )
    from concourse.dve_spec import C1 as C1L, C2 as C2L
    from concourse.dve_table_gen import DveOpSpec

    name = "HC_GATE3_RED"
    if name in CUSTOM_DVE_SPECS:
        return next(o for o in OPS if o.name == name)

    yc = maxx(minn(Src0, C0), Zero - C0)
    body = ((sq(yc) * C1L + C2L) * yc) * Src1

    def _ref(in0, in1, s0, s1, imm2):
        x = np.clip(np.asarray(in0, np.float32), -s0, s0)
        w = np.asarray(in1, np.float32)
        out = ((x * x * s1 + imm2) * x) * w
        return out, out.sum(axis=-1, keepdims=True)

    spec = Spec(body=body, accum=AluOp.ADD, reference=_ref)
    row = _CUSTOM_DVE_ROW_BASE + len(OPS)
    _SUB_OPCODE_FOR_NAME[name] = row
    shas = {}
    for ver in ("v3", "v4"):
        dspec = DveOpSpec(name=name, opcode=row, uops=lower(spec, ver=ver),
                          rd1_en=True)
        shas[ver] = dspec.sha(ver)
    dve_op = DveOp(name, spec, subdim=False, uops_sha=shas)
    OPS.append(dve_op)
    CUSTOM_DVE_SPECS[name] = spec
    return dve_op


def _build_nc():
    if "nc" in _CACHE:
        return _CACHE["nc"]

    dve_op = _register_custom_op()

    nc = bacc.Bacc(
        "TRN2",
        target_bir_lowering=False,
        debug=False,
        num_devices=N_CORES,
    )
    u_d = nc.dram_tensor("u", [B, O_SH, IN], F32, kind="ExternalInput").ap()
    x16_d = nc.dram_tensor("x16", [B, IN], F16, kind="ExternalInput").ap()
    w12_d = nc.dram_tensor("w12", [O_SH, IN], F16, kind="ExternalInput").ap()
    la16_d = nc.dram_tensor("la16", [O_SH, IN], F16, kind="ExternalInput").ap()
    wt16_d = nc.dram_tensor("wt16", [IN, O_SH], F16, kind="ExternalInput").ap()
    xt16_d = nc.dram_tensor("xt16", [IN, B], F16, kind="ExternalInput").ap()
    bias_d = nc.dram_tensor("bias", [O_SH], F32, kind="ExternalInput").ap()
    out_d = nc.dram_tensor("out", [128, H, B], F32, kind="ExternalOutput").ap()

    with tile.TileContext(nc) as tc, ExitStack() as ctx:
        _kernel_body(ctx, tc, dve_op, u_d, x16_d, w12_d, la16_d, wt16_d,
                     xt16_d, bias_d, out_d)

    nc.compile()
    _CACHE["nc"] = nc
    return nc


def _bcast_row(ap_row):
    """[1, n] AP -> [128, n] AP with 0 partition stride."""
    return bass.AP(
        tensor=ap_row.tensor,
        offset=ap_row.offset,
        ap=[[0, 128], list(ap_row.ap[-1])],
    )


def _kernel_body(ctx, tc, dve_op, u_d, x16_d, w12_d, la16_d, wt16_d, xt16_d,
                 bias_d, out_d):
    nc = tc.nc
    Ln = mybir.ActivationFunctionType.Ln
    op = mybir.AluOpType

    singles = ctx.enter_context(tc.tile_pool(name="singles", bufs=1))

    # --- main loop pools (declared before constants so the first u/xb DMAs
    # are issued ahead of the setup DMAs on the in-order DMA queue) ---
    upool = ctx.enter_context(tc.tile_pool(name="u", bufs=3))
    l2pool = ctx.enter_context(tc.tile_pool(name="l2", bufs=2))
    ycpool = ctx.enter_context(tc.tile_pool(name="yc", bufs=2))
    xbpool = ctx.enter_context(tc.tile_pool(name="xb", bufs=2))
    cbpool = ctx.enter_context(tc.tile_pool(name="cb", bufs=4))
    junkpool = ctx.enter_context(tc.tile_pool(name="junk", bufs=1))

    # first working-set DMAs go out first; batch 0 is loaded per half so the
    # first ACT/DVE work starts after ~3us instead of ~6us of DMA
    ut0 = upool.tile([128, H, IN], F32)
    la16 = singles.tile([128, H, IN], F16)
    w12 = singles.tile([128, H, IN], F16)
    xb0 = xbpool.tile([128, IN], F16)
    la_v = la16_d.rearrange("(h p) i -> p h i", p=128)
    w12_v = w12_d.rearrange("(h p) i -> p h i", p=128)
    # dependency-ordered first transfers: everything the first (b=0, h=0)
    # unit needs, then the h=1 half, then the rest
    nc.sync.dma_start(out=ut0[:, 0, :], in_=u_d[0, 0:128, :])
    nc.sync.dma_start(out=la16[:, 0, :], in_=la_v[:, 0, :])
    nc.sync.dma_start(out=xb0, in_=_bcast_row(x16_d[0:1, :]))
    nc.sync.dma_start(out=w12[:, 0, :], in_=w12_v[:, 0, :])
    nc.sync.dma_start(out=ut0[:, 1, :], in_=u_d[0, 128:256, :])
    nc.sync.dma_start(out=la16[:, 1, :], in_=la_v[:, 1, :])
    nc.sync.dma_start(out=w12[:, 1, :], in_=w12_v[:, 1, :])
    bias_col = singles.tile([128, H], F32)
    nc.sync.dma_start(out=bias_col, in_=bias_d.rearrange("(h p) -> p h", p=128))

    # accumulator strip, column index = h*B + b
    accM = singles.tile([128, H * B], F32)
    s0 = singles.tile([128, H * B], F32)

    def unit(b, ut, xb):
        l2 = l2pool.tile([128, H, IN], F16)
        yc = ycpool.tile([128, H, IN], F16)
        if b == 0:
            # per-half for a shorter pipeline-fill ramp
            for h in range(H):
                nc.scalar.activation(l2[:, h, :], ut[:, h, :], Ln,
                                     bias=1.0, scale=-1.0)
                nc.vector.tensor_sub(l2[:, h, :], la16[:, h, :], l2[:, h, :])
                nc.scalar.activation(yc[:, h, :], ut[:, h, :], Ln)
                nc.vector.tensor_add(yc[:, h, :], yc[:, h, :], l2[:, h, :])
        else:
            nc.scalar.activation(l2, ut, Ln, bias=1.0, scale=-1.0)   # ln(1-u)
            # DVE: y1 = la - l2 (in place, plain tensor_tensor runs at 2x)
            nc.vector.tensor_sub(l2, la16, l2)
            nc.scalar.activation(yc, ut, Ln)                         # ln(u)
            nc.vector.tensor_add(yc, yc, l2)                         # y
        for h in range(H):
            cb = cbpool.tile([128, IN], F16)
            nc.gpsimd.tensor_mul(cb, w12[:, h, :], xb)           # Pool
            junk = junkpool.tile([128, IN], F16)
            col = h * B + b
            nc.vector._custom_dve(
                dve_op,
                out=junk,
                in0=yc[:, h, :],
                in1=cb,
                s0=C_CLAMP, s1=C3, imm2=C1,
                accum_out=accM[:, col:col + 1],
            )

    for b in range(B):
        ut = ut0 if b == 0 else upool.tile([128, H, IN], F32)
        if b > 0:
            nc.sync.dma_start(out=ut, in_=u_d[b].rearrange("(h p) i -> p h i", p=128))
        xb = xb0 if b == 0 else xbpool.tile([128, IN], F16)
        if b > 0:
            nc.sync.dma_start(out=xb, in_=_bcast_row(x16_d[b:b + 1, :]))
        unit(b, ut, xb)

    # --- S0 via PE: S0[o, b] = sum_i w[o,i] x[b,i] (feeds only the combine,
    # so it is emitted last and fills engine idle time near the tail) ---
    wt = singles.tile([128, IN // 128, O_SH], F16)
    nc.sync.dma_start(out=wt, in_=wt16_d.rearrange("(ki p) o -> p ki o", p=128))
    xt = singles.tile([128, IN // 128, B], F16)
    nc.sync.dma_start(out=xt, in_=xt16_d.rearrange("(ki p) b -> p ki b", p=128))
    with tc.psum_pool(name="ps", bufs=2) as psp:
        for h in range(H):
            osl = slice(h * 128, (h + 1) * 128)
            pm = psp.tile([128, B], F32)
            for ki in range(IN // 128):
                nc.tensor.matmul(pm, wt[:, ki, osl], xt[:, ki, :],
                                 start=(ki == 0), stop=(ki == IN // 128 - 1))
            nc.vector.tensor_copy(s0[:, h * B:(h + 1) * B], pm)

    # --- final combine: out = accM + 0.5*S0 + bias ---
    comb = singles.tile([128, H * B], F32)
    nc.vector.scalar_tensor_tensor(out=comb, in0=s0, scalar=0.5,
                                   in1=accM, op0=op.mult, op1=op.add)
    for h in range(H):
        nc.vector.tensor_scalar(
            comb[:, h * B:(h + 1) * B], comb[:, h * B:(h + 1) * B],
            bias_col[:, h:h + 1], None, op.add,
        )
    out_v = out_d.rearrange("p h b -> p (h b)")
    nc.sync.dma_start(out=out_v, in_=comb)


def kernel(x, u, weight, log_alpha, bias):
    x = np.ascontiguousarray(x, dtype=np.float32)
    u = np.ascontiguousarray(u, dtype=np.float32)
    weight = np.ascontiguousarray(weight, dtype=np.float32)
    log_alpha = np.ascontiguousarray(log_alpha, dtype=np.float32)
    bias = np.ascontiguousarray(bias, dtype=np.float32)

    nc = _build_nc()

    x16 = x.astype(np.float16)
    in_maps = []
    for c in range(N_CORES):
        sl = slice(c * O_SH, (c + 1) * O_SH)
        wsl = weight[sl]
        in_maps.append(
            {
                "u": np.ascontiguousarray(u[:, sl, :]),
                "x16": x16,
                "w12": np.ascontiguousarray((1.2 * wsl).astype(np.float16)),
                "la16": np.ascontiguousarray(log_alpha[sl].astype(np.float16)),
                "wt16": np.ascontiguousarray(wsl.T.astype(np.float16)),
                "xt16": np.ascontiguousarray(x.T.astype(np.float16)),
                "bias": np.ascontiguousarray(bias[sl]),
            }
        )

    import os

    trace = bool(int(os.environ.get("KERNEL_TRACE", "0")))
    res = run_bass_kernel_spmd(
        nc, in_maps, core_ids=list(range(N_CORES)), trace=trace
    )
    kernel._last = res

    out = np.empty((B, OUT), dtype=np.float32)
    for c in range(N_CORES):
        oc = res.results[c]["out"]          # [128, H, B]
        out[:, c * O_SH:(c + 1) * O_SH] = oc.transpose(2, 1, 0).reshape(B, O_SH)
    return out


# revision 24
# speedup vs baseline: 1.7313x; 1.0048x over previous
"""Trainium2 Bass kernel for L0-regularized linear forward (hard-concrete gate).

Computes out[b,o] = sum_i x[b,i] * W[o,i] * z[b,o,i] + bias[o]
  where s = sigmoid((log(u) - log1p(-u) + log_alpha) / (2/3))
        z = clip(s * 1.2 - 0.1, 0, 1)

Shapes: x[32,2048] u[32,2048,2048] W[2048,2048] la[2048,2048] bias[2048]
Sharding: output-dim sharded, 2048/8 = 256 rows per core; no collectives.

Math used on device:
  y  = ln(u) - ln(1-u) + la;  yc = clamp(y, +-c), c = ln(11)/1.5
  z  = 1.2*sigmoid(1.5*yc) - 0.1              (exact: clip folds into clamp)
  sigmoid(1.5*y)-0.5 ~= p(y) = (c3*y^2 + c1)*y (deg-3, coefficients optimized
  for end-to-end variance against u~U(0,1); rel out err ~6e-3 vs 2e-2 gate)
  out = sum_i p(yc)*cb + 0.5*S0 + bias   with cb = 1.2*w*x, S0 = sum_i w*x

Engine schedule per (batch, half-of-256-rows) unit, all f16 after ACT:
  DMA : u half-tile (f32), broadcast row of x (f16)
  ACT : l2 = ln(1-u), l1 = ln(u)       (only the natural_log table -> 1 load)
  Pool: y1 = la - l2                   (scalar_tensor_tensor, eff 0.6)
  DVE : y = l1 + y1; custom op HC_GATE3_RED = clamp+poly3+mul-cb+accumulate
        in ONE instruction; cb = w12*xb (some offloaded to Pool)
  PE  : S0 row-sums of w*x (tiny f16 matmuls into PSUM)
"""

import sys
from contextlib import ExitStack

import numpy as np

if "/opt/trn_rl_repo" not in sys.path:
    sys.path.insert(0, "/opt/trn_rl_repo")

import concourse.bass as bass
import concourse.tile as tile
from concourse import bacc, mybir
from concourse.bass_utils import run_bass_kernel_spmd

F32 = mybir.dt.float32
F16 = mybir.dt.float16

B, OUT, IN = 32, 2048, 2048
N_CORES = 8
O_SH = OUT // N_CORES          # 256 output rows per core
H = O_SH // 128                # 2 partition-halves per core

C_CLAMP = float(np.log(11.0) / 1.5)
# variance-optimal deg-3: sigmoid(1.5 y) - 0.5 ~ (C3 y^2 + C1) y on [-c, c]
C1, C3 = 0.358500, -0.038292

N_CB_POOL = 3                  # of every 8 cb passes, this many go to Pool

_CACHE = {}


def _register_custom_op():
    """One fused DVE instruction:
        yc  = clamp(in0, -s0, s0)
        out = ((yc^2 * s1 + imm2) * yc) * in1
        accum_out = sum(out, free axis)
    8 ALU stages exactly (Zero-C0 is hoisted as stream-invariant)."""
    from concourse.dve_ops import (
        CUSTOM_DVE_SPECS,
        OPS,
        _CUSTOM_DVE_ROW_BASE,
        _SUB_OPCODE_FOR_NAME,
        DveOp,
    )
    from concourse.dve_spec import (
        AluOp, C0, Spec, Src0, Src1, Zero, lower, maxx, minn, sq,
    )
    from concourse.dve_spec import C1 as C1L, C2 as C2L
    from concourse.dve_table_gen import DveOpSpec

    name = "HC_GATE3_RED"
    if name in CUSTOM_DVE_SPECS:
        return next(o for o in OPS if o.name == name)

    yc = maxx(minn(Src0, C0), Zero - C0)
    body = ((sq(yc) * C1L + C2L) * yc) * Src1

    def _ref(in0, in1, s0, s1, imm2):
        x = np.clip(np.asarray(in0, np.float32), -s0, s0)
        w = np.asarray(in1, np.float32)
        out = ((x * x * s1 + imm2) * x) * w
        return out, out.sum(axis=-1, keepdims=True)

    spec = Spec(body=body, accum=AluOp.ADD, reference=_ref)
    row = _CUSTOM_DVE_ROW_BASE + len(OPS)
    _SUB_OPCODE_FOR_NAME[name] = row
    shas = {}
    for ver in ("v3", "v4"):
        dspec = DveOpSpec(name=name, opcode=row, uops=lower(spec, ver=ver),
                          rd1_en=True)
        shas[ver] = dspec.sha(ver)
    dve_op = DveOp(name, spec, subdim=False, uops_sha=shas)
    OPS.append(dve_op)
    CUSTOM_DVE_SPECS[name] = spec
    return dve_op


def _build_nc():
    if "nc" in _CACHE:
        return _CACHE["nc"]

    dve_op = _register_custom_op()

    nc = bacc.Bacc(
        "TRN2",
        target_bir_lowering=False,
        debug=False,
        num_devices=N_CORES,
    )
    u_d = nc.dram_tensor("u", [B, O_SH, IN], F32, kind="ExternalInput").ap()
    x16_d = nc.dram_tensor("x16", [B, IN], F16, kind="ExternalInput").ap()
    w12_d = nc.dram_tensor("w12", [O_SH, IN], F16, kind="ExternalInput").ap()
    la16_d = nc.dram_tensor("la16", [O_SH, IN], F16, kind="ExternalInput").ap()
    wt16_d = nc.dram_tensor("wt16", [IN, O_SH], F16, kind="ExternalInput").ap()
    xt16_d = nc.dram_tensor("xt16", [IN, B], F16, kind="ExternalInput").ap()
    bias_d = nc.dram_tensor("bias", [O_SH], F32, kind="ExternalInput").ap()
    out_d = nc.dram_tensor("out", [128, H, B], F32, kind="ExternalOutput").ap()

    with tile.TileContext(nc) as tc, ExitStack() as ctx:
        _kernel_body(ctx, tc, dve_op, u_d, x16_d, w12_d, la16_d, wt16_d,
                     xt16_d, bias_d, out_d)

    nc.compile()
    _CACHE["nc"] = nc
    return nc


def _bcast_row(ap_row):
    """[1, n] AP -> [128, n] AP with 0 partition stride."""
    return bass.AP(
        tensor=ap_row.tensor,
        offset=ap_row.offset,
        ap=[[0, 128], list(ap_row.ap[-1])],
    )


def _kernel_body(ctx, tc, dve_op, u_d, x16_d, w12_d, la16_d, wt16_d, xt16_d,
                 bias_d, out_d):
    nc = tc.nc
    Ln = mybir.ActivationFunctionType.Ln
    op = mybir.AluOpType

    singles = ctx.enter_context(tc.tile_pool(name="singles", bufs=1))

    # --- main loop pools (declared before constants so the first u/xb DMAs
    # are issued ahead of the setup DMAs on the in-order DMA queue) ---
    upool = ctx.enter_context(tc.tile_pool(name="u", bufs=3))
    l2pool = ctx.enter_context(tc.tile_pool(name="l2", bufs=2))
    ycpool = ctx.enter_context(tc.tile_pool(name="yc", bufs=2))
    xbpool = ctx.enter_context(tc.tile_pool(name="xb", bufs=2))
    cbpool = ctx.enter_context(tc.tile_pool(name="cb", bufs=4))
    junkpool = ctx.enter_context(tc.tile_pool(name="junk", bufs=1))

    # first working-set DMAs go out first; batch 0 is loaded per half so the
    # first ACT/DVE work starts after ~3us instead of ~6us of DMA
    ut0 = upool.tile([128, H, IN], F32)
    la16 = singles.tile([128, H, IN], F16)
    w12 = singles.tile([128, H, IN], F16)
    xb0 = xbpool.tile([128, IN], F16)
    la_v = la16_d.rearrange("(h p) i -> p h i", p=128)
    w12_v = w12_d.rearrange("(h p) i -> p h i", p=128)
    # dependency-ordered first transfers at quarter granularity: everything
    # the first (b=0, h=0, q=0) work needs, then successive quarters
    Q0 = IN // 2
    nc.sync.dma_start(out=ut0[:, 0, :Q0], in_=u_d[0, 0:128, :Q0])
    nc.sync.dma_start(out=la16[:, 0, :Q0], in_=la_v[:, 0, :Q0])
    nc.sync.dma_start(out=xb0[:, :Q0], in_=_bcast_row(x16_d[0:1, :Q0]))
    nc.sync.dma_start(out=w12[:, 0, :Q0], in_=w12_v[:, 0, :Q0])
    nc.sync.dma_start(out=ut0[:, 0, Q0:], in_=u_d[0, 0:128, Q0:])
    nc.sync.dma_start(out=la16[:, 0, Q0:], in_=la_v[:, 0, Q0:])
    nc.sync.dma_start(out=xb0[:, Q0:], in_=_bcast_row(x16_d[0:1, Q0:]))
    nc.sync.dma_start(out=w12[:, 0, Q0:], in_=w12_v[:, 0, Q0:])
    nc.sync.dma_start(out=ut0[:, 1, :], in_=u_d[0, 128:256, :])
    nc.sync.dma_start(out=la16[:, 1, :], in_=la_v[:, 1, :])
    nc.sync.dma_start(out=w12[:, 1, :], in_=w12_v[:, 1, :])
    bias_col = singles.tile([128, H], F32)
    nc.sync.dma_start(out=bias_col, in_=bias_d.rearrange("(h p) -> p h", p=128))

    # accumulator strip, column index = h*B + b
    accM = singles.tile([128, H * B], F32)
    accX = singles.tile([128, H], F32)      # b=0 second-column-half partials
    s0 = singles.tile([128, H * B], F32)

    def unit(b, ut, xb):
        l2 = l2pool.tile([128, H, IN], F16)
        yc = ycpool.tile([128, H, IN], F16)
        if b == 1:
            # half-grain: b1's second u half is still in flight when its
            # first half is ready to process
            for h in range(H):
                nc.scalar.activation(l2[:, h, :], ut[:, h, :], Ln,
                                     bias=1.0, scale=-1.0)
                nc.vector.tensor_sub(l2[:, h, :], la16[:, h, :], l2[:, h, :])
                nc.scalar.activation(yc[:, h, :], ut[:, h, :], Ln)
                nc.vector.tensor_add(yc[:, h, :], yc[:, h, :], l2[:, h, :])
        else:
            nc.scalar.activation(l2, ut, Ln, bias=1.0, scale=-1.0)  # ln(1-u)
            # DVE: y1 = la - l2 (in place, plain tensor_tensor at 2x)
            nc.vector.tensor_sub(l2, la16, l2)
            nc.scalar.activation(yc, ut, Ln)                        # ln(u)
            nc.vector.tensor_add(yc, yc, l2)                        # y
        for h in range(H):
            cb = cbpool.tile([128, IN], F16)
            nc.gpsimd.tensor_mul(cb, w12[:, h, :], xb)           # Pool
            junk = junkpool.tile([128, IN], F16)
            col = h * B + b
            nc.vector._custom_dve(
                dve_op,
                out=junk,
                in0=yc[:, h, :],
                in1=cb,
                s0=C_CLAMP, s1=C3, imm2=C1,
                accum_out=accM[:, col:col + 1],
            )

    def unit0(ut, xb):
        # batch 0 runs at quarter granularity so every engine starts ~5us
        # earlier during pipeline fill; the second column-half accumulates
        # into accX and is folded in at the combine
        Q = IN // 2
        l2 = l2pool.tile([128, H, IN], F16)
        yc = ycpool.tile([128, H, IN], F16)
        for h in range(H):
            cb = cbpool.tile([128, IN], F16)
            junk = junkpool.tile([128, IN], F16)
            for q in range(2):
                cs = slice(q * Q, (q + 1) * Q)
                nc.scalar.activation(l2[:, h, cs], ut[:, h, cs], Ln,
                                     bias=1.0, scale=-1.0)
                nc.vector.tensor_sub(l2[:, h, cs], la16[:, h, cs], l2[:, h, cs])
                nc.scalar.activation(yc[:, h, cs], ut[:, h, cs], Ln)
                nc.vector.tensor_add(yc[:, h, cs], yc[:, h, cs], l2[:, h, cs])
                nc.gpsimd.tensor_mul(cb[:, cs], w12[:, h, cs], xb[:, cs])
                acc = accM[:, h * B:h * B + 1] if q == 0 else accX[:, h:h + 1]
                nc.vector._custom_dve(
                    dve_op,
                    out=junk[:, cs],
                    in0=yc[:, h, cs],
                    in1=cb[:, cs],
                    s0=C_CLAMP, s1=C3, imm2=C1,
                    accum_out=acc,
                )

    for b in range(B):
        if b == 0:
            unit0(ut0, xb0)
            continue
        ut = upool.tile([128, H, IN], F32)
        xb = xbpool.tile([128, IN], F16)
        if b == 1:
            nc.sync.dma_start(out=ut[:, 0, :], in_=u_d[b, 0:128, :])
            nc.sync.dma_start(out=xb, in_=_bcast_row(x16_d[b:b + 1, :]))
            nc.sync.dma_start(out=ut[:, 1, :], in_=u_d[b, 128:256, :])
        else:
            nc.sync.dma_start(out=ut, in_=u_d[b].rearrange("(h p) i -> p h i", p=128))
            nc.sync.dma_start(out=xb, in_=_bcast_row(x16_d[b:b + 1, :]))
        unit(b, ut, xb)

    # --- S0 via PE: S0[o, b] = sum_i w[o,i] x[b,i] (feeds only the combine,
    # so it is emitted last and fills engine idle time near the tail) ---
    wt = singles.tile([128, IN // 128, O_SH], F16)
    nc.sync.dma_start(out=wt, in_=wt16_d.rearrange("(ki p) o -> p ki o", p=128))
    xt = singles.tile([128, IN // 128, B], F16)
    nc.sync.dma_start(out=xt, in_=xt16_d.rearrange("(ki p) b -> p ki b", p=128))
    with tc.psum_pool(name="ps", bufs=2) as psp:
        for h in range(H):
            osl = slice(h * 128, (h + 1) * 128)
            pm = psp.tile([128, B], F32)
            for ki in range(IN // 128):
                nc.tensor.matmul(pm, wt[:, ki, osl], xt[:, ki, :],
                                 start=(ki == 0), stop=(ki == IN // 128 - 1))
            nc.vector.tensor_copy(s0[:, h * B:(h + 1) * B], pm)

    # --- final combine: out = accM (+ accX for b=0) + 0.5*S0 + bias ---
    for h in range(H):
        nc.vector.tensor_add(accM[:, h * B:h * B + 1],
                             accM[:, h * B:h * B + 1], accX[:, h:h + 1])
    comb = singles.tile([128, H * B], F32)
    nc.vector.scalar_tensor_tensor(out=comb, in0=s0, scalar=0.5,
                                   in1=accM, op0=op.mult, op1=op.add)
    for h in range(H):
        nc.vector.tensor_scalar(
            comb[:, h * B:(h + 1) * B], comb[:, h * B:(h + 1) * B],
            bias_col[:, h:h + 1], None, op.add,
        )
    out_v = out_d.rearrange("p h b -> p (h b)")
    nc.sync.dma_start(out=out_v, in_=comb)


def kernel(x, u, weight, log_alpha, bias):
    x = np.ascontiguousarray(x, dtype=np.float32)
    u = np.ascontiguousarray(u, dtype=np.float32)
    weight = np.ascontiguousarray(weight, dtype=np.float32)
    log_alpha = np.ascontiguousarray(log_alpha, dtype=np.float32)
    bias = np.ascontiguousarray(bias, dtype=np.float32)

    nc = _build_nc()

    x16 = x.astype(np.float16)
    in_maps = []
    for c in range(N_CORES):
        sl = slice(c * O_SH, (c + 1) * O_SH)
        wsl = weight[sl]
        in_maps.append(
            {
                "u": np.ascontiguousarray(u[:, sl, :]),
                "x16": x16,
                "w12": np.ascontiguousarray((1.2 * wsl).astype(np.float16)),
                "la16": np.ascontiguousarray(log_alpha[sl].astype(np.float16)),
                "wt16": np.ascontiguousarray(wsl.T.astype(np.float16)),
                "xt16": np.ascontiguousarray(x.T.astype(np.float16)),
                "bias": np.ascontiguousarray(bias[sl]),
            }
        )

    import os

    trace = bool(int(os.environ.get("KERNEL_TRACE", "0")))
    res = run_bass_kernel_spmd(
        nc, in_maps, core_ids=list(range(N_CORES)), trace=trace
    )
    kernel._last = res

    out = np.empty((B, OUT), dtype=np.float32)
    for c in range(N_CORES):
        oc = res.results[c]["out"]          # [128, H, B]
        out[:, c * O_SH:(c + 1) * O_SH] = oc.transpose(2, 1, 0).reshape(B, O_SH)
    return out
